# revision 11
# baseline (speedup 1.0000x reference)
"""Trainium2 Bass kernel for nn_MinimalBeatDecoder (nms_detection), v3.

Reference semantics: peaks = positive local maxima of a 7-wide window over a
16.7M-frame logit stream; output = frame index of each peak (sections are
single peaks in the no-tie case), first 2^21 of them, padded with -1.

Per core (2^21 frames as 128 rows x 16384), chunked [512, 1536, 2048 x 7]:
  - ACT engine casts each fp32 chunk into two dense bf16 relu planes
    (even/odd frames) via strided activation ops. relu folds the x > 0
    test into the window max; relu+bf16 rounding is monotone, so the bf16
    comparisons yield a candidate-peak SUPERSET of the true fp32 peaks
    (bf16 ties and all-nonpositive plateaus add ~0.5% false candidates).
  - DVE computes the 7-window candidate mask on the planes with 7 dense
    2x-mode tensor_tensor ops per chunk:
      P[s]  = max(xe[s], xo[s])            pair max
      R[s]  = max(P[s-1], P[s+1])
      Q[s]  = max(R[s], P[s])              6-window max
      eM[s] = xe[s] >= max(Q[s], xo[s-2])  even-parity candidates
      oM[s] = xo[s] >= max(Q[s], xe[s+2])  odd-parity candidates
  - the masks stream straight back to DRAM (no on-device compaction: the
    only compaction engine, GPSIMD LocalScatter, runs at ~3ns/idx and
    starves the DVE while active, costing more than it saves).
  - host: decode masks to ordered candidate positions (vectorized, at most
    2 per pair), exact fp32 verification of every candidate (vectorized
    7-window max at candidate positions) removes the false candidates.

An exact numpy fallback handles inputs with adjacent-equal fp32 peak ties
(reference merges those into averaged sections; gaussian inputs never tie).
"""

import sys

sys.path.insert(0, "/opt/trn_rl_repo")

import numpy as np

import concourse.bacc as bacc
import concourse.bass as bass
import concourse.mybir as mybir
import concourse.tile as tile
from concourse import bass_utils

# geometry
NCORES = 8
NFRAMES = 16_777_216
PERCORE = NFRAMES // NCORES  # 2^21
MAX_BEATS = NFRAMES // 8  # 2^21

P = 128
W = PERCORE // P  # 16384 frames per row
WP = W // 2  # 8192 pairs per row
HALO = 8

CHUNKS = [512, 1024, 2048, 4096, 4096, 2048, 1024, 1024, 512]

F32 = mybir.dt.float32
BF16 = mybir.dt.bfloat16
I16 = mybir.dt.int16
I8 = mybir.dt.int8


def build_kernel(p=P, w=W):
    """Per-core SPMD program. Inputs:
      xin [p*w + HALO] f32  (frame t of this core at index t+4)
    Outputs:
      me [p, WP] i16  (even-parity candidate mask, 1 bit per pair)
      mo [p, WP] i16  (odd-parity candidate mask)
    """
    nc = bacc.Bacc("TRN2", target_bir_lowering=False)
    xin = nc.dram_tensor("xin", [p * w + HALO], F32, kind="ExternalInput")
    me_d = nc.dram_tensor("me", [p, WP], I16, kind="ExternalOutput")
    mo_d = nc.dram_tensor("mo", [p, WP], I16, kind="ExternalOutput")

    MX = mybir.AluOpType.max
    GE = mybir.AluOpType.is_ge
    RELU = mybir.ActivationFunctionType.Relu

    with tile.TileContext(nc) as tc:
        with (
            tc.tile_pool(name="io", bufs=4) as io_pool,
            tc.tile_pool(name="pl", bufs=3) as pl_pool,
            tc.tile_pool(name="wk", bufs=2) as wk_pool,
        ):
            o = 0  # pair offset within row
            for j, cw in enumerate(CHUNKS):
                hw = cw // 2
                off = 2 * o
                xh = io_pool.tile([p, 4104], F32, tag="xh")
                src = bass.AP(tensor=xin, offset=off, ap=[[w, p], [1, cw + 8]])
                nc.sync.dma_start(xh[:, 0 : cw + 8], src)

                # relu bf16 planes: xet[k]=relu(x[2(k-2)]), xot[k]=relu(x[2(k-2)+1])
                xet = pl_pool.tile([p, 2052], BF16, tag="xet")
                xot = pl_pool.tile([p, 2052], BF16, tag="xot")
                nc.scalar.activation(
                    xet[:, 0 : hw + 4], xh[:, 0 : cw + 8 : 2], RELU, bias=0.0
                )
                nc.scalar.activation(
                    xot[:, 0 : hw + 4], xh[:, 1 : cw + 8 : 2], RELU, bias=0.0
                )

                Pt = wk_pool.tile([p, 2052], BF16, tag="Pt")
                nc.vector.tensor_tensor(
                    out=Pt[:, 0 : hw + 4], in0=xet[:, 0 : hw + 4],
                    in1=xot[:, 0 : hw + 4], op=MX,
                )
                Rt = wk_pool.tile([p, 2048], BF16, tag="Rt")
                nc.vector.tensor_tensor(
                    out=Rt[:, 0:hw], in0=Pt[:, 1 : hw + 1],
                    in1=Pt[:, 3 : hw + 3], op=MX,
                )
                Qt = wk_pool.tile([p, 2048], BF16, tag="Qt")
                nc.vector.tensor_tensor(
                    out=Qt[:, 0:hw], in0=Rt[:, 0:hw], in1=Pt[:, 2 : hw + 2],
                    op=MX,
                )
                eW = wk_pool.tile([p, 2048], BF16, tag="eW")
                nc.vector.tensor_tensor(
                    out=eW[:, 0:hw], in0=Qt[:, 0:hw], in1=xot[:, 0:hw], op=MX
                )
                eM = wk_pool.tile([p, 2048], I16, tag="eM")
                nc.vector.tensor_tensor(
                    out=eM[:, 0:hw], in0=xet[:, 2 : hw + 2], in1=eW[:, 0:hw],
                    op=GE,
                )
                oW = wk_pool.tile([p, 2048], BF16, tag="oW")
                nc.vector.tensor_tensor(
                    out=oW[:, 0:hw], in0=Qt[:, 0:hw], in1=xet[:, 4 : hw + 4],
                    op=MX,
                )
                oM = wk_pool.tile([p, 2048], I16, tag="oM")
                nc.vector.tensor_tensor(
                    out=oM[:, 0:hw], in0=xot[:, 2 : hw + 2], in1=oW[:, 0:hw],
                    op=GE,
                )

                nc.sync.dma_start(me_d[:, o : o + hw], eM[:, 0:hw])
                nc.sync.dma_start(mo_d[:, o : o + hw], oM[:, 0:hw])
                o += hw
    nc.compile()
    return nc


_cached = {}


def _get_nc():
    if "nc" not in _cached:
        _cached["nc"] = build_kernel()
    return _cached["nc"]


def _host_reference_fallback(x):
    """Exact numpy fallback (used only for adjacent-equal fp32 peak ties)."""
    n = x.shape[0]
    import numpy.lib.stride_tricks as st

    xp = np.pad(x, (3, 3), constant_values=-np.inf)
    pooled = st.sliding_window_view(xp, 7).max(axis=1)
    peak = (x == pooled) & (x > 0)
    idx = np.arange(n, dtype=np.int64)
    prev = np.concatenate([[False], peak[:-1]])
    is_new = peak & ~prev
    sec = np.cumsum(is_new) - 1
    sums = np.zeros(MAX_BEATS + 1, np.float64)
    cnts = np.zeros(MAX_BEATS + 1, np.float64)
    sel = peak & (sec < MAX_BEATS)
    np.add.at(sums, sec[sel], idx[sel].astype(np.float64))
    np.add.at(cnts, sec[sel], 1.0)
    out = np.full(MAX_BEATS, -1.0, np.float32)
    m = cnts[:MAX_BEATS] > 0
    out[m] = (sums[:MAX_BEATS][m] / cnts[:MAX_BEATS][m]).astype(np.float32)
    return out[None, :]


def kernel(logit: np.ndarray) -> np.ndarray:
    x = np.asarray(logit, dtype=np.float32)[0]

    # host guard: adjacent-equal fp32 window maxima need the exact path
    eq_next = x[:-1] == x[1:]
    if eq_next.any():
        cand = np.nonzero(eq_next)[0]
        cand = cand[(x[cand] > 0)]
        if cand.size:
            xp = np.pad(x, (3, 3), constant_values=-np.inf)
            for i in cand:
                if (
                    x[i] == xp[i : i + 7].max()
                    and x[i + 1] == xp[i + 1 : i + 8].max()
                ):
                    return _host_reference_fallback(x)

    nc = _get_nc()

    xpad = np.full(NFRAMES + 8, np.float32(-3.0e38), dtype=np.float32)
    xpad[4 : 4 + NFRAMES] = x

    in_maps = []
    for c in range(NCORES):
        base = c * PERCORE
        in_maps.append(
            {"xin": np.ascontiguousarray(xpad[base : base + PERCORE + HALO])}
        )

    global _last_in_maps
    _last_in_maps = in_maps
    res = bass_utils.run_bass_kernel_spmd(nc, in_maps, core_ids=list(range(NCORES)))

    # host: masks -> ordered candidate positions (<= 2 per pair, even first)
    em = np.concatenate([res.results[c]["me"].reshape(-1) for c in range(NCORES)])
    om = np.concatenate([res.results[c]["mo"].reshape(-1) for c in range(NCORES)])
    v = em + 2 * om  # flat pair index == global pair (row-major == frame order)
    nz = np.flatnonzero(v)
    vv = v[nz]
    both = vv == 3
    ncand = nz.size + int(both.sum())
    first = 2 * nz + (vv == 2)  # even position unless odd-only
    starts = np.cumsum(1 + both) - (1 + both)
    cand = np.empty(ncand, dtype=np.int64)
    cand[starts] = first
    cand[starts[both] + 1] = 2 * nz[both] + 1

    # exact fp32 verification of every candidate (removes bf16/relu ties)
    xg = np.pad(x, (3, 3), constant_values=-np.float32(np.inf))
    win = xg[cand[:, None] + np.arange(7)[None, :]]
    xv = x[cand]
    keep = (xv >= win.max(axis=1)) & (xv > 0)
    beats = cand[keep][:MAX_BEATS]

    out = np.full(MAX_BEATS, -1.0, dtype=np.float32)
    out[: beats.size] = beats.astype(np.float32)
    return out[None, :]


# revision 12
# speedup vs baseline: 1.1415x; 1.1415x over previous
"""Trainium2 Bass kernel for nn_MinimalBeatDecoder (nms_detection), v3.

Reference semantics: peaks = positive local maxima of a 7-wide window over a
16.7M-frame logit stream; output = frame index of each peak (sections are
single peaks in the no-tie case), first 2^21 of them, padded with -1.

Per core (2^21 frames as 128 rows x 16384), chunked [512, 1536, 2048 x 7]:
  - ACT engine casts each fp32 chunk into two dense bf16 relu planes
    (even/odd frames) via strided activation ops. relu folds the x > 0
    test into the window max; relu+bf16 rounding is monotone, so the bf16
    comparisons yield a candidate-peak SUPERSET of the true fp32 peaks
    (bf16 ties and all-nonpositive plateaus add ~0.5% false candidates).
  - DVE computes the 7-window candidate mask on the planes with 7 dense
    2x-mode tensor_tensor ops per chunk:
      P[s]  = max(xe[s], xo[s])            pair max
      R[s]  = max(P[s-1], P[s+1])
      Q[s]  = max(R[s], P[s])              6-window max
      eM[s] = xe[s] >= max(Q[s], xo[s-2])  even-parity candidates
      oM[s] = xo[s] >= max(Q[s], xe[s+2])  odd-parity candidates
  - the masks stream straight back to DRAM (no on-device compaction: the
    only compaction engine, GPSIMD LocalScatter, runs at ~3ns/idx and
    starves the DVE while active, costing more than it saves).
  - host: decode masks to ordered candidate positions (vectorized, at most
    2 per pair), exact fp32 verification of every candidate (vectorized
    7-window max at candidate positions) removes the false candidates.

An exact numpy fallback handles inputs with adjacent-equal fp32 peak ties
(reference merges those into averaged sections; gaussian inputs never tie).
"""

import sys

sys.path.insert(0, "/opt/trn_rl_repo")

import numpy as np

import concourse.bacc as bacc
import concourse.bass as bass
import concourse.mybir as mybir
import concourse.tile as tile
from concourse import bass_utils

# geometry
NCORES = 8
NFRAMES = 16_777_216
PERCORE = NFRAMES // NCORES  # 2^21
MAX_BEATS = NFRAMES // 8  # 2^21

P = 128
W = PERCORE // P  # 16384 frames per row
WP = W // 2  # 8192 pairs per row
HALO = 8

CHUNKS = [512, 1536, 4096, 4096, 4096, 1536, 512]

F32 = mybir.dt.float32
BF16 = mybir.dt.bfloat16
I16 = mybir.dt.int16
I8 = mybir.dt.int8


def build_kernel(p=P, w=W):
    """Per-core SPMD program. Inputs:
      xin [p*w + HALO] f32  (frame t of this core at index t+4)
    Outputs:
      me [p, WP] i16  (even-parity candidate mask, 1 bit per pair)
      mo [p, WP] i16  (odd-parity candidate mask)
    """
    nc = bacc.Bacc("TRN2", target_bir_lowering=False)
    xin = nc.dram_tensor("xin", [p * w + HALO], F32, kind="ExternalInput")
    me_d = nc.dram_tensor("me", [p, WP], I16, kind="ExternalOutput")
    mo_d = nc.dram_tensor("mo", [p, WP], I16, kind="ExternalOutput")

    MX = mybir.AluOpType.max
    GE = mybir.AluOpType.is_ge
    RELU = mybir.ActivationFunctionType.Relu

    with tile.TileContext(nc) as tc:
        with (
            tc.tile_pool(name="io", bufs=4) as io_pool,
            tc.tile_pool(name="pl", bufs=3) as pl_pool,
            tc.tile_pool(name="wk", bufs=2) as wk_pool,
        ):
            o = 0  # pair offset within row
            for j, cw in enumerate(CHUNKS):
                hw = cw // 2
                off = 2 * o
                xh = io_pool.tile([p, 4104], F32, tag="xh")
                src = bass.AP(tensor=xin, offset=off, ap=[[w, p], [1, cw + 8]])
                nc.sync.dma_start(xh[:, 0 : cw + 8], src)

                # relu bf16 planes: xet[k]=relu(x[2(k-2)]), xot[k]=relu(x[2(k-2)+1])
                xet = pl_pool.tile([p, 2052], BF16, tag="xet")
                xot = pl_pool.tile([p, 2052], BF16, tag="xot")
                nc.scalar.activation(
                    xet[:, 0 : hw + 4], xh[:, 0 : cw + 8 : 2], RELU, bias=0.0
                )
                nc.scalar.activation(
                    xot[:, 0 : hw + 4], xh[:, 1 : cw + 8 : 2], RELU, bias=0.0
                )

                Pt = wk_pool.tile([p, 2052], BF16, tag="Pt")
                nc.vector.tensor_tensor(
                    out=Pt[:, 0 : hw + 4], in0=xet[:, 0 : hw + 4],
                    in1=xot[:, 0 : hw + 4], op=MX,
                )
                Rt = wk_pool.tile([p, 2048], BF16, tag="Rt")
                nc.vector.tensor_tensor(
                    out=Rt[:, 0:hw], in0=Pt[:, 1 : hw + 1],
                    in1=Pt[:, 3 : hw + 3], op=MX,
                )
                Qt = wk_pool.tile([p, 2048], BF16, tag="Qt")
                nc.vector.tensor_tensor(
                    out=Qt[:, 0:hw], in0=Rt[:, 0:hw], in1=Pt[:, 2 : hw + 2],
                    op=MX,
                )
                # candidates vs the 6-window max only: a SUPERSET of the
                # true 7-window masks (~17% extra); host verification is
                # exact, so the two per-parity edge comparisons are free to
                # skip on device.
                eM = wk_pool.tile([p, 2048], I16, tag="eM")
                nc.vector.tensor_tensor(
                    out=eM[:, 0:hw], in0=xet[:, 2 : hw + 2], in1=Qt[:, 0:hw],
                    op=GE,
                )
                oM = wk_pool.tile([p, 2048], I16, tag="oM")
                nc.vector.tensor_tensor(
                    out=oM[:, 0:hw], in0=xot[:, 2 : hw + 2], in1=Qt[:, 0:hw],
                    op=GE,
                )

                nc.sync.dma_start(me_d[:, o : o + hw], eM[:, 0:hw])
                nc.sync.dma_start(mo_d[:, o : o + hw], oM[:, 0:hw])
                o += hw
    nc.compile()
    return nc


_cached = {}


def _get_nc():
    if "nc" not in _cached:
        _cached["nc"] = build_kernel()
    return _cached["nc"]


def _host_reference_fallback(x):
    """Exact numpy fallback (used only for adjacent-equal fp32 peak ties)."""
    n = x.shape[0]
    import numpy.lib.stride_tricks as st

    xp = np.pad(x, (3, 3), constant_values=-np.inf)
    pooled = st.sliding_window_view(xp, 7).max(axis=1)
    peak = (x == pooled) & (x > 0)
    idx = np.arange(n, dtype=np.int64)
    prev = np.concatenate([[False], peak[:-1]])
    is_new = peak & ~prev
    sec = np.cumsum(is_new) - 1
    sums = np.zeros(MAX_BEATS + 1, np.float64)
    cnts = np.zeros(MAX_BEATS + 1, np.float64)
    sel = peak & (sec < MAX_BEATS)
    np.add.at(sums, sec[sel], idx[sel].astype(np.float64))
    np.add.at(cnts, sec[sel], 1.0)
    out = np.full(MAX_BEATS, -1.0, np.float32)
    m = cnts[:MAX_BEATS] > 0
    out[m] = (sums[:MAX_BEATS][m] / cnts[:MAX_BEATS][m]).astype(np.float32)
    return out[None, :]


def kernel(logit: np.ndarray) -> np.ndarray:
    x = np.asarray(logit, dtype=np.float32)[0]

    # host guard: adjacent-equal fp32 window maxima need the exact path
    eq_next = x[:-1] == x[1:]
    if eq_next.any():
        cand = np.nonzero(eq_next)[0]
        cand = cand[(x[cand] > 0)]
        if cand.size:
            xp = np.pad(x, (3, 3), constant_values=-np.inf)
            for i in cand:
                if (
                    x[i] == xp[i : i + 7].max()
                    and x[i + 1] == xp[i + 1 : i + 8].max()
                ):
                    return _host_reference_fallback(x)

    nc = _get_nc()

    xpad = np.full(NFRAMES + 8, np.float32(-3.0e38), dtype=np.float32)
    xpad[4 : 4 + NFRAMES] = x

    in_maps = []
    for c in range(NCORES):
        base = c * PERCORE
        in_maps.append(
            {"xin": np.ascontiguousarray(xpad[base : base + PERCORE + HALO])}
        )

    global _last_in_maps
    _last_in_maps = in_maps
    res = bass_utils.run_bass_kernel_spmd(nc, in_maps, core_ids=list(range(NCORES)))

    # host: masks -> ordered candidate positions (<= 2 per pair, even first)
    em = np.concatenate([res.results[c]["me"].reshape(-1) for c in range(NCORES)])
    om = np.concatenate([res.results[c]["mo"].reshape(-1) for c in range(NCORES)])
    v = em + 2 * om  # flat pair index == global pair (row-major == frame order)
    nz = np.flatnonzero(v)
    vv = v[nz]
    both = vv == 3
    ncand = nz.size + int(both.sum())
    first = 2 * nz + (vv == 2)  # even position unless odd-only
    starts = np.cumsum(1 + both) - (1 + both)
    cand = np.empty(ncand, dtype=np.int64)
    cand[starts] = first
    cand[starts[both] + 1] = 2 * nz[both] + 1

    # exact fp32 verification of every candidate (removes bf16/relu ties)
    xg = np.pad(x, (3, 3), constant_values=-np.float32(np.inf))
    win = xg[cand[:, None] + np.arange(7)[None, :]]
    xv = x[cand]
    keep = (xv >= win.max(axis=1)) & (xv > 0)
    beats = cand[keep][:MAX_BEATS]

    out = np.full(MAX_BEATS, -1.0, dtype=np.float32)
    out[: beats.size] = beats.astype(np.float32)
    return out[None, :]
